# revision 3
# baseline (speedup 1.0000x reference)
"""Trainium2 Bass kernel for greedy sequential independent-set sampling.

Reference semantics: sites visited in row-major order; site (r, c) is set to 1
iff u[s, r, c] < 0.5 and no already-set lattice neighbor. Because the visit
order is row-major, right/down neighbors are still 0 when a site is decided:

    x[r, c] = (u[r, c] < 0.5) & ~x[r-1, c] & ~x[r, c-1]

One DVE tensor_tensor_scan per lattice row computes the whole thing:

    state' = (c[w] - state) is_gt x_prev[w]      (op0=subtract, op1=is_gt)

where c is either sign(0.5 - u) in {-1,+1} (ScalarE Sign) or (u < 0.5) in
{0,1} (Pool is_lt) -- both give identical scan results -- and state carries
x[r, c-1]. Scans are DVE-only (the scan opcode is not implemented on Pool).

Sample axis is data-parallel: 65536 samples -> 8 cores x 8192 samples; per
core 64 groups of 128 samples (SBUF partition dim). Groups are packed side
by side in the scan free dim with one dummy column (c = 0) per group so the
carried state resets at group boundaries.

Engine split:
  - DVE: all 32 row scans (the serial critical path).
  - Compare (u < 0.5) alternates per input slab between ScalarE (Sign) and
    Pool (is_lt) into per-slab tiles, keeping both off the DVE.
  - Output stays int8 (values 0/1) incl. dummy cols; the host drops dummies
    and upcasts to int32 (4x less output DMA than shipping int32).
"""

import numpy as np

import concourse.bacc as bacc
import concourse.mybir as mybir
from concourse.tile import TileContext
from concourse.bass_utils import run_bass_kernel_spmd

N_CORES = 8
S_TOTAL = 65536
R = 32
C = 32
P = 128  # SBUF partitions

SPC = S_TOTAL // N_CORES  # samples per core: 8192
G = SPC // P  # 64 groups of 128 samples
W = C + 1  # 33: one dummy col per group resets the scan carry
L = G * W  # 2112 bytes per lattice row per partition

SLABS = [(0, 2), (2, 2), (4, 4), (8, 4), (12, 4), (16, 4), (20, 4), (24, 4), (28, 4)]

F32 = mybir.dt.float32
I8 = mybir.dt.int8


def build_nc():
    """Build the per-core Bass program (SPMD: same program, different data)."""
    nc = bacc.Bacc("TRN2", target_bir_lowering=False, debug=False)
    # Host-permuted input: the top byte of each fp32 u value (u < 0.5 iff
    # byte3 < 0x3F for u in [0,1)), laid out [p][r][g][w] with a 0x7F
    # dummy byte at w=32 of each group.
    u = nc.declare_dram_parameter("u", [P, R * L], I8, isOutput=False)
    # Output keeps the dummy cols (the host drops them): every DMA
    # descriptor is a single >=2KB contiguous run per partition.
    cfg = nc.declare_dram_parameter("config", [P, R * L], I8, isOutput=True)

    with TileContext(nc) as tc:
        with (
            tc.tile_pool(name="const", bufs=1) as constp,
            tc.tile_pool(name="b", bufs=3) as bp,
            tc.tile_pool(name="c", bufs=4) as cp,
            tc.tile_pool(name="x", bufs=1) as xp,
        ):
            thr = constp.tile([P, 1], F32, tag="thr")
            nc.gpsimd.memset(thr[:], 62.5)

            # x slot 0 = virtual lattice row -1 (all zeros)
            x = xp.tile([P, (R + 1) * L], I8, tag="x")
            nc.gpsimd.memset(x[:, 0:L], 0)

            # Output blocks in x-slot space (slot s holds lattice row s-1).
            # A block [s0, s0+nb) may only be DMA'd once the scan front is
            # LAG slots past it: the DMA's reads must stay a full SBUF bank
            # (2 KiB) behind the next scan's write, or Tile's bank-level
            # tracking serializes the scan chain behind the DMA.
            blocks = [(s, 4) for s in range(1, 29, 4)] + [(29, 2), (31, 1), (32, 1)]
            LAG = 2
            bi = 0

            def flush(front):
                # front = highest x slot whose scan has been issued
                nonlocal bi
                while bi < len(blocks):
                    s0, nb = blocks[bi]
                    if front < 33 and s0 + nb + LAG > front:
                        break
                    nc.sync.dma_start(
                        out=cfg[:, (s0 - 1) * L : (s0 - 1 + nb) * L],
                        in_=x[:, s0 * L : (s0 + nb) * L],
                    )
                    bi += 1

            for si, (a, nr) in enumerate(SLABS):
                b = bp.tile([P, 4 * L], I8, tag="b")
                seg = b[:, 0 : nr * L]
                nc.sync.dma_start(out=seg, in_=u[:, a * L : (a + nr) * L])

                # compare: c nonzero iff u < 0.5 (byte3 < 63); dummy byte
                # 0x7F compares false, giving the carry-reset value.
                c = cp.tile([P, 4 * L], I8, tag="c")
                cseg = c[:, 0 : nr * L]
                if si % 2 == 0:
                    nc.scalar.activation(
                        out=cseg,
                        in_=seg,
                        func=mybir.ActivationFunctionType.Sign,
                        bias=thr[:],
                        scale=-1.0,
                    )
                else:
                    nc.gpsimd.tensor_scalar(
                        out=cseg,
                        in0=seg,
                        scalar1=63,
                        scalar2=None,
                        op0=mybir.AluOpType.is_lt,
                    )

                for j in range(nr):
                    r = a + j
                    # state' = (c - state) > x_up : the full site update
                    nc.vector.tensor_tensor_scan(
                        out=x[:, (r + 1) * L : (r + 2) * L],
                        data0=c[:, j * L : (j + 1) * L],
                        data1=x[:, r * L : (r + 1) * L],
                        initial=0.0,
                        op0=mybir.AluOpType.subtract,
                        op1=mybir.AluOpType.is_gt,
                    )
                    flush(r + 1)
            flush(33)
    nc.compile()
    return nc


def host_permute_u(u_core):
    """[spc, 32, 32] f32 -> top-byte plane [P, R*G*W] int8.

    For u in [0, 1), u < 0.5 iff the fp32 top byte (sign + exp[7:1]) is
    < 0x3F; only that byte is shipped to the device (4x less input DMA).
    """
    b3 = u_core.reshape(-1).view(np.uint8)[3::4]
    v = b3.reshape(G, P, R, C).transpose(1, 2, 0, 3)  # [p, r, g, c]
    out = np.full((P, R, G, W), 0x7F, np.uint8)
    out[..., :C] = v
    return out.view(np.int8).reshape(P, R * L)


def host_unpermute_cfg(res):
    """{config: [P, R*G*W] int8} -> [spc, 32, 32] int32."""
    v = res["config"].reshape(P, R, G, W)[..., :C]  # [p, r, g, c]
    # sample s = g*P + p
    return (
        np.ascontiguousarray(v.transpose(2, 0, 1, 3))
        .reshape(SPC, R, C)
        .astype(np.int32)
    )


_NC_CACHE = {}


def _get_nc():
    if "nc" not in _NC_CACHE:
        _NC_CACHE["nc"] = build_nc()
    return _NC_CACHE["nc"]


def kernel(u, n_rows=32, n_cols=32, **_):
    u = np.ascontiguousarray(np.asarray(u), dtype=np.float32)
    assert u.shape == (S_TOTAL, R, C), u.shape
    assert int(n_rows) == R and int(n_cols) == C

    nc = _get_nc()
    in_maps = [
        {"u": host_permute_u(u[i * SPC : (i + 1) * SPC])} for i in range(N_CORES)
    ]
    res = run_bass_kernel_spmd(nc, in_maps, list(range(N_CORES)))
    out = np.concatenate(
        [host_unpermute_cfg(res.results[i]) for i in range(N_CORES)], axis=0
    )
    return out.reshape(S_TOTAL, R, C)


# revision 4
# speedup vs baseline: 5.0166x; 5.0166x over previous
"""Trainium2 Bass kernel for greedy sequential independent-set sampling.

Reference semantics: sites visited in row-major order; site (r, c) is set to 1
iff u[s, r, c] < 0.5 and no already-set lattice neighbor. Because the visit
order is row-major, right/down neighbors are still 0 when a site is decided:

    x[r, c] = (u[r, c] < 0.5) & ~x[r-1, c] & ~x[r, c-1]

One DVE tensor_tensor_scan per lattice row computes the whole thing:

    state' = (nb[w] - state) is_gt x_prev[w]     (op0=subtract, op1=is_gt)

where nb = sign(0.5 - u) in {-1,+1} (ScalarE Sign) and state carries
x[r, c-1].

Measured rates (HW microbench): the scan runs at ~2.15 ns/elem regardless of
dtype (2-op feedback loop, no DVE fast mode applies), ScalarE activation at
~0.87 ns/elem, Pool tensor ops are 4-16 ns/elem (useless).  So: DVE does all
32 chained row scans (the 146 us critical path), ScalarE does the whole
compare pass (58 us, runs ahead), Pool does nothing, and all DMA (8.65 MB in,
8.65 MB out per core) hides behind the scan chain.

Sample axis is data-parallel: 65536 samples -> 8 cores x 8192 samples; per
core 64 groups of 128 samples (SBUF partition dim). Groups are packed side
by side in the scan free dim with one dummy column (nb = -1) per group so
the carried state resets at group boundaries.

Output stays int8 (values 0/1) incl. dummy cols; the host drops dummies and
upcasts to int32 (4x less output DMA than shipping int32).
"""

import numpy as np

import concourse.bacc as bacc
import concourse.mybir as mybir
from concourse.tile import TileContext
from concourse.bass_utils import run_bass_kernel_spmd

N_CORES = 8
S_TOTAL = 65536
R = 32
C = 32
P = 128  # SBUF partitions

SPC = S_TOTAL // N_CORES  # samples per core: 8192
G = SPC // P  # 64 groups of 128 samples
W = C + 1  # 33: one dummy col per group resets the scan carry
L = G * W  # 2112 bytes per lattice row per partition

# First slabs small so the scan chain starts ~2.5 us in; steady state 4-row.
SLABS = [(0, 1), (1, 1), (2, 2), (4, 4), (8, 4), (12, 4), (16, 4), (20, 4),
         (24, 4), (28, 4)]

F32 = mybir.dt.float32
I8 = mybir.dt.int8


def build_nc():
    """Build the per-core Bass program (SPMD: same program, different data)."""
    nc = bacc.Bacc("TRN2", target_bir_lowering=False, debug=False)
    # Host-permuted input: the top byte of each fp32 u value (u < 0.5 iff
    # byte3 < 0x3F for u in [0,1)), laid out [p][r][g][w] with a 0x7F
    # dummy byte at w=32 of each group.
    u = nc.declare_dram_parameter("u", [P, R * L], I8, isOutput=False)
    # Output keeps the dummy cols (the host drops them): every DMA
    # descriptor is a single >=2KB contiguous run per partition.
    cfg = nc.declare_dram_parameter("config", [P, R * L], I8, isOutput=True)

    with TileContext(nc) as tc:
        with (
            tc.tile_pool(name="const", bufs=1) as constp,
            tc.tile_pool(name="b", bufs=3) as bp,
            tc.tile_pool(name="c", bufs=3) as cp,
            tc.tile_pool(name="x", bufs=1) as xp,
        ):
            thr = constp.tile([P, 1], F32, tag="thr")
            nc.gpsimd.memset(thr[:], 62.5)

            # x slot 0 = virtual lattice row -1 (all zeros).  DVE memset:
            # it overlaps the input-DMA + first-compare head, so it is free.
            x = xp.tile([P, (R + 1) * L], I8, tag="x")
            nc.vector.memset(x[:, 0:L], 0)

            # Output blocks in x-slot space (slot s holds lattice row s-1).
            # A block [s0, s0+nb) may only be DMA'd once the scan front is
            # LAG slots past it: the DMA's reads must stay a full SBUF bank
            # (2 KiB) behind the next scan's write, or Tile's bank-level
            # tracking serializes the scan chain behind the DMA.
            blocks = [(s, 4) for s in range(1, 29, 4)] + [(29, 2), (31, 1), (32, 1)]
            LAG = 2
            bi = 0

            def flush(front):
                # front = highest x slot whose scan has been issued
                nonlocal bi
                while bi < len(blocks):
                    s0, nb = blocks[bi]
                    if front < 33 and s0 + nb + LAG > front:
                        break
                    nc.sync.dma_start(
                        out=cfg[:, (s0 - 1) * L : (s0 - 1 + nb) * L],
                        in_=x[:, s0 * L : (s0 + nb) * L],
                    )
                    bi += 1

            for a, nr in SLABS:
                b = bp.tile([P, 4 * L], I8, tag="b")
                seg = b[:, 0 : nr * L]
                nc.sync.dma_start(out=seg, in_=u[:, a * L : (a + nr) * L])

                # nb = sign(62.5 - byte3) in {-1, +1}: +1 iff u < 0.5; the
                # 0x7F dummy bytes come out as the -1 carry reset.
                c = cp.tile([P, 4 * L], I8, tag="c")
                cseg = c[:, 0 : nr * L]
                nc.scalar.activation(
                    out=cseg,
                    in_=seg,
                    func=mybir.ActivationFunctionType.Sign,
                    bias=thr[:],
                    scale=-1.0,
                )

                for j in range(nr):
                    r = a + j
                    # state' = (nb - state) > x_up : the full site update
                    nc.vector.tensor_tensor_scan(
                        out=x[:, (r + 1) * L : (r + 2) * L],
                        data0=c[:, j * L : (j + 1) * L],
                        data1=x[:, r * L : (r + 1) * L],
                        initial=0.0,
                        op0=mybir.AluOpType.subtract,
                        op1=mybir.AluOpType.is_gt,
                    )
                    flush(r + 1)
            flush(33)
    nc.compile()
    return nc


def host_permute_u(u_core):
    """[spc, 32, 32] f32 -> top-byte plane [P, R*G*W] int8.

    For u in [0, 1), u < 0.5 iff the fp32 top byte (sign + exp[7:1]) is
    < 0x3F; only that byte is shipped to the device (4x less input DMA).
    """
    b3 = u_core.reshape(-1).view(np.uint8)[3::4]
    v = b3.reshape(G, P, R, C).transpose(1, 2, 0, 3)  # [p, r, g, c]
    out = np.full((P, R, G, W), 0x7F, np.uint8)
    out[..., :C] = v
    return out.view(np.int8).reshape(P, R * L)


def host_unpermute_cfg(res):
    """{config: [P, R*G*W] int8} -> [spc, 32, 32] int32."""
    v = res["config"].reshape(P, R, G, W)[..., :C]  # [p, r, g, c]
    # sample s = g*P + p
    return (
        np.ascontiguousarray(v.transpose(2, 0, 1, 3))
        .reshape(SPC, R, C)
        .astype(np.int32)
    )


_NC_CACHE = {}


def _get_nc():
    if "nc" not in _NC_CACHE:
        _NC_CACHE["nc"] = build_nc()
    return _NC_CACHE["nc"]


def kernel(u, n_rows=32, n_cols=32, **_):
    u = np.ascontiguousarray(np.asarray(u), dtype=np.float32)
    assert u.shape == (S_TOTAL, R, C), u.shape
    assert int(n_rows) == R and int(n_cols) == C

    nc = _get_nc()
    in_maps = [
        {"u": host_permute_u(u[i * SPC : (i + 1) * SPC])} for i in range(N_CORES)
    ]
    res = run_bass_kernel_spmd(nc, in_maps, list(range(N_CORES)))
    out = np.concatenate(
        [host_unpermute_cfg(res.results[i]) for i in range(N_CORES)], axis=0
    )
    return out.reshape(S_TOTAL, R, C)
